# revision 3
# baseline (speedup 1.0000x reference)
"""nn_Attention Trainium2 Bass kernel — data-parallel over batch on 8 NeuronCores.

Per core (one batch element) the reference computes
  qh = q@Wq + bq; kh = k@Wk + bk; vh = kh@Wv + bv
  scores = qh@kh.T (+ mask -> -10000); probs = softmax(scores)
  out = (probs @ vh) @ Wo + bo

Algebraic restructuring (host precomputes tiny [D,D] products in fp64):
  scores  = q @ (Wq Wk^T) @ k^T + colbias[j] (+ row-const terms that softmax
            drops), colbias = k @ (Wk bq)
  probs @ vh @ Wo = probs @ (k @ (Wk Wv Wo)) + bo2,  bo2 = (bk Wv + bv) Wo + bo
            (exact: probs rows sum to 1)
Mask exploitation: masked k columns get probs == 0 exactly, so the host packs
only the unmasked k columns (padded with zero columns + -10000 bias to a
multiple of 256). For a ~half-dense random mask this cuts the scores/AV/value
work by ~2x.

Device (per core), Kp = padded packed-k count, chunks of <=512 k cols:
  Akt[d,j]  = At.T-tiles @ ktp          (At = (Wq Wk^T)^T)          [f32r]
  vhw[l,v]  = ktp.T-tiles @ B3          (B3 = Wk Wv Wo)             [bf16]
  per q-tile: scores = qt.T-tiles @ Akt  + ones.T @ biasrow (rank-1 in PSUM)
              softmax rowwise over chunks (max, exp with accum-sum, recip)
              probsT via PE transposes (bf16)
  outT[v,q] = vhw-tiles.T @ probsT      (+bo2 per-partition)        [f32]
Host: out[b] = outT.T
"""
import numpy as np

import concourse.bass as bass
import concourse.mybir as mybir
from concourse import bacc, tile
from concourse.bass_utils import run_bass_kernel_spmd
from concourse.masks import make_identity

B, L, D = 8, 2048, 1024
P = 128
F32 = mybir.dt.float32
F32R = mybir.dt.float32r
BF16 = mybir.dt.bfloat16
AF = mybir.ActivationFunctionType
AX = mybir.AxisListType

QBLK = 512          # q columns per outer block
NQB = L // QBLK     # 4
DT = D // P         # 8 d tiles


def build_nc(Kp):
    KT = Kp // P
    chunks = [(s, min(512, Kp - s)) for s in range(0, Kp, 512)]
    NCH = len(chunks)

    nc = bacc.Bacc("TRN2", target_bir_lowering=False, debug=False, num_devices=8)
    qt_d = nc.dram_tensor("qt", [D, L], F32R, kind="ExternalInput").ap()
    ktp_d = nc.dram_tensor("ktp", [D, Kp], F32R, kind="ExternalInput").ap()
    at_d = nc.dram_tensor("at", [D, D], F32R, kind="ExternalInput").ap()
    b3_d = nc.dram_tensor("b3", [D, D], F32R, kind="ExternalInput").ap()
    biasrow_d = nc.dram_tensor("biasrow", [1, Kp], F32R, kind="ExternalInput").ap()
    bo2_d = nc.dram_tensor("bo2", [D, 1], F32, kind="ExternalInput").ap()
    ones_d = nc.dram_tensor("ones", [1, P], F32R, kind="ExternalInput").ap()
    out_d = nc.dram_tensor("out", [D, L], F32, kind="ExternalOutput").ap()

    with tile.TileContext(nc) as tc:
        with tc.tile_pool(name="const", bufs=1) as cp, \
             tc.tile_pool(name="persist", bufs=1) as pp_:
            bo2_t = cp.tile([P, DT], F32)
            for i in range(DT):
                nc.gpsimd.dma_start(out=bo2_t[:, i:i + 1], in_=bo2_d[i * P:(i + 1) * P, :])
            onesr_t = cp.tile([1, P], F32R)
            nc.gpsimd.dma_start(out=onesr_t, in_=ones_d)
            biasrow_t = cp.tile([1, Kp], F32R)
            nc.gpsimd.dma_start(out=biasrow_t, in_=biasrow_d)
            ident_f = cp.tile([P, P], F32)
            make_identity(nc, ident_f)
            identb_t = cp.tile([P, P], BF16)
            nc.vector.tensor_copy(identb_t, ident_f)

            akt = pp_.tile([P, DT, Kp], F32R, tag="akt")
            vhw = pp_.tile([P, KT, D], BF16, tag="vhw")

            # early prefetch of first q-block
            fq_cm = tc.tile_pool(name="fq", bufs=1, side="right")
            fqp = fq_cm.__enter__()
            first_q = fqp.tile([P, DT, QBLK], F32R, tag="fq")

            # ---------- phase A: Akt = At.T @ ktp ; vhw = ktp.T @ B3
            with tc.tile_pool(name="aw", bufs=1) as awp, \
                 tc.tile_pool(name="aps", bufs=4, space="PSUM") as app:
                at_t = awp.tile([P, DT, D], F32R, tag="at")
                b3_t = awp.tile([P, DT, D], F32R, tag="b3")
                ktp_t = awp.tile([P, DT, Kp], F32R, tag="ktp")
                for i in range(DT):
                    nc.sync.dma_start(out=ktp_t[:, i], in_=ktp_d[i * P:(i + 1) * P, :])
                    nc.scalar.dma_start(out=at_t[:, i], in_=at_d[i * P:(i + 1) * P, :])
                    nc.gpsimd.dma_start(out=b3_t[:, i], in_=b3_d[i * P:(i + 1) * P, :])
                for d in range(DT):
                    nc.scalar.dma_start(out=first_q[:, d], in_=qt_d[d * P:(d + 1) * P, 0:QBLK])

                for dt in range(DT):
                    for (c0, w) in chunks:
                        ps = app.tile([P, 512], F32, tag="ps")
                        for dp in range(DT):
                            nc.tensor.matmul(ps[:, :w], at_t[:, dp, dt * P:(dt + 1) * P],
                                             ktp_t[:, dp, c0:c0 + w],
                                             start=(dp == 0), stop=(dp == DT - 1))
                        nc.scalar.activation(akt[:, dt, c0:c0 + w], ps[:, :w], AF.Copy)
                for lt in range(KT):
                    for vc in range(0, D, 512):
                        ps = app.tile([P, 512], F32, tag="ps")
                        for d in range(DT):
                            nc.tensor.matmul(ps, ktp_t[:, d, lt * P:(lt + 1) * P],
                                             b3_t[:, d, vc:vc + 512],
                                             start=(d == 0), stop=(d == DT - 1))
                        nc.scalar.activation(vhw[:, lt, vc:vc + 512], ps, AF.Copy)

            # ---------- phase B: attention per q-block
            with tc.tile_pool(name="bq", bufs=2) as qp, \
                 tc.tile_pool(name="bpt", bufs=1) as ptp, \
                 tc.tile_pool(name="bp", bufs=2) as ppool, \
                 tc.tile_pool(name="bsm", bufs=3) as smp, \
                 tc.tile_pool(name="bst", bufs=2) as stp, \
                 tc.tile_pool(name="bps_s", bufs=4, space="PSUM") as pss, \
                 tc.tile_pool(name="bps_t", bufs=2, space="PSUM") as pst, \
                 tc.tile_pool(name="bps_m", bufs=2, space="PSUM") as psm:
                probsT = ptp.tile([P, KT, QBLK], BF16, tag="probsT")

                def emit_transposes(qt_i, p_t):
                    for kt in range(KT):
                        tp = pst.tile([P, P], BF16, tag="tp")
                        nc.tensor.transpose(tp, p_t[:, kt * P:(kt + 1) * P], identb_t)
                        nc.vector.tensor_copy(probsT[:, kt, qt_i * P:(qt_i + 1) * P], tp)

                qtbs = {0: first_q}
                for qb in range(NQB):
                    qtb = qtbs.pop(qb)
                    if qb + 1 < NQB:
                        nxq = qp.tile([P, DT, QBLK], F32R, tag="qtb")
                        for d in range(DT):
                            nc.scalar.dma_start(
                                out=nxq[:, d],
                                in_=qt_d[d * P:(d + 1) * P,
                                         (qb + 1) * QBLK:(qb + 2) * QBLK])
                        qtbs[qb + 1] = nxq

                    p_prev = None
                    for qt_i in range(QBLK // P):
                        scs = []
                        for ci, (c0, w) in enumerate(chunks):
                            ps = pss.tile([P, 512], F32, tag="sc")
                            for d in range(DT):
                                nc.tensor.matmul(ps[:, :w],
                                                 qtb[:, d, qt_i * P:(qt_i + 1) * P],
                                                 akt[:, d, c0:c0 + w],
                                                 start=(d == 0), stop=False)
                            nc.tensor.matmul(ps[:, :w], onesr_t,
                                             biasrow_t[:, c0:c0 + w],
                                             start=False, stop=True)
                            scs.append(ps)
                        mx3 = smp.tile([P, 4], F32, tag="mx3")
                        for ci, (c0, w) in enumerate(chunks):
                            nc.vector.reduce_max(mx3[:, ci:ci + 1], scs[ci][:, :w],
                                                 axis=AX.X)
                        mx = smp.tile([P, 1], F32, tag="mx")
                        nc.vector.reduce_max(mx, mx3[:, 0:NCH], axis=AX.X)
                        negmx = smp.tile([P, 1], F32, tag="negmx")
                        nc.vector.tensor_scalar_mul(negmx, mx, -1.0)
                        p_t = ppool.tile([P, Kp], BF16, tag="p")
                        s3 = smp.tile([P, 4], F32, tag="s3")
                        for ci, (c0, w) in enumerate(chunks):
                            nc.scalar.activation(p_t[:, c0:c0 + w], scs[ci][:, :w],
                                                 AF.Exp, bias=negmx,
                                                 accum_out=s3[:, ci:ci + 1])
                        sume = smp.tile([P, 1], F32, tag="sume")
                        nc.vector.reduce_sum(sume, s3[:, 0:NCH], axis=AX.X)
                        recip = smp.tile([P, 1], F32, tag="recip")
                        nc.vector.reciprocal(recip, sume)
                        nc.vector.tensor_scalar_mul(p_t, p_t, recip)
                        if p_prev is not None:
                            emit_transposes(qt_i - 1, p_prev)
                        p_prev = p_t
                    emit_transposes(QBLK // P - 1, p_prev)

                    for vt in range(DT):
                        ps = psm.tile([P, QBLK], F32, tag="avps")
                        for kt in range(KT):
                            nc.tensor.matmul(ps, vhw[:, kt, vt * P:(vt + 1) * P],
                                             probsT[:, kt], start=(kt == 0),
                                             stop=(kt == KT - 1))
                        ot = stp.tile([P, QBLK], F32, tag="ot")
                        nc.scalar.activation(ot, ps, AF.Identity,
                                             bias=bo2_t[:, vt:vt + 1])
                        nc.sync.dma_start(
                            out=out_d[vt * P:(vt + 1) * P, qb * QBLK:(qb + 1) * QBLK],
                            in_=ot)
                    if qb == 0:
                        fq_cm.__exit__(None, None, None)
    nc.compile()
    return nc


_NC_CACHE = {}


def _get_nc(Kp):
    if Kp not in _NC_CACHE:
        _NC_CACHE[Kp] = build_nc(Kp)
    return _NC_CACHE[Kp]


def build_in_maps(q, k, mask, Wq, bq, Wk, bk, Wv, bv, Wo, bo):
    q = np.asarray(q, np.float32)
    k = np.asarray(k, np.float32)
    mask = np.asarray(mask)
    Wq64, Wk64, Wv64, Wo64 = (np.asarray(w, np.float64)
                              for w in (Wq, Wk, Wv, Wo))
    bq64, bk64, bv64, bo64 = (np.asarray(v, np.float64)
                              for v in (bq, bk, bv, bo))

    At = np.ascontiguousarray((Wk64 @ Wq64.T).astype(np.float32))   # (Wq Wk^T)^T
    B3 = np.ascontiguousarray((Wk64 @ Wv64 @ Wo64).astype(np.float32))
    u = Wk64 @ bq64                                                 # colbias = kp @ u
    bo2 = ((bk64 @ Wv64 + bv64) @ Wo64 + bo64).astype(np.float32)
    ones = np.ones((1, P), np.float32)

    idxs = [np.nonzero(np.asarray(mask)[b])[0] for b in range(B)]
    sizes = [(len(ix) if len(ix) else L) for ix in idxs]
    Kp = max(256, ((max(sizes) + 255) // 256) * 256)

    in_maps = []
    for b in range(B):
        idx = idxs[b]
        biasrow = np.full((1, Kp), -10000.0, np.float32)
        ktp = np.zeros((D, Kp), np.float32)
        if len(idx):
            n = len(idx)
            kp_rows = k[b][idx]                      # [n, D]
            ktp[:, :n] = kp_rows.T
            biasrow[0, :n] = (kp_rows.astype(np.float64) @ u).astype(np.float32)
        else:
            # degenerate all-masked batch: keep every column, bias them equally
            ktp[:, :L] = k[b].T
            biasrow[0, :L] = (k[b].astype(np.float64) @ u).astype(np.float32) - 10000.0
        in_maps.append({
            "qt": np.ascontiguousarray(q[b].T),
            "ktp": ktp,
            "at": At, "b3": B3,
            "biasrow": biasrow,
            "bo2": bo2.reshape(D, 1),
            "ones": ones,
        })
    return in_maps, Kp


def kernel(q, k, mask, Wq, bq, Wk, bk, Wv, bv, Wo, bo):
    in_maps, Kp = build_in_maps(q, k, mask, Wq, bq, Wk, bk, Wv, bv, Wo, bo)
    nc = _get_nc(Kp)
    res = run_bass_kernel_spmd(nc, in_maps, core_ids=list(range(B)))
    out = np.stack([np.ascontiguousarray(res.results[b]["out"].T) for b in range(B)])
    return out.astype(np.float32)
